# revision 27
# baseline (speedup 1.0000x reference)
"""NLBlockND (embedded-gaussian non-local block, 2D, bn_layer=True) on 8 TRN2 cores.

Strategy (see spec sharding hint): data-parallel over batch N=4, x2
sequence-parallel over the 4096 query tokens -> 8 shards of (batch n,
query half h). Each core gets the full 512x4096 x for its batch with the
key/query axis ROTATED so its 2048 queries are always columns 0..2047
(attention is permutation-invariant over keys, so rotating the key axis
changes nothing). Per core:

  theta = theta_w^T-proj of x[:, :2048] + tb          [256, 2048]  (f32r matmul)
  phi   = phi_w^T-proj of x + pb                      [256, 4096]
  gT    = x^T-proj against g_w^T (transposed layout)  [4096, 256] (+ ones col)
  S^T   = phi_tile^T @ theta  (keys on partitions)    [4096, 512] per q-group
  P^T   = exp(S^T)            (no max-sub needed: |logit| <= ~20)
  y^T   = P^T-tiles^T @ [gT | 1]  -> cols 0..255 = unnormalized y^T,
          col 256 = softmax denominator s[q]  (free row-sum trick)
  y^T  *= 1/s[q]  (per-partition scale fused into PSUM->SBUF copy)
  y     = PE-transpose(y^T)
  w_y   = wz_w^T-proj of y + (wz_b + wz_w @ g_b)      [512, 2048]
          (g_b is folded into the wz bias on the host: attn rows sum to 1)
  per-channel mean/var of w_y slice via bn_stats/bn_aggr

Host: gather w_y slices, combine slice stats into global batch-norm
stats, normalize, affine, residual-add x.

All big matmuls run as float32r (fp32 bits, full PE rate at N>=256).
"""

import sys

if "/opt/trn_rl_repo" not in sys.path:
    sys.path.insert(0, "/opt/trn_rl_repo")

from contextlib import ExitStack

import numpy as np

import concourse.bass as bass
import concourse.tile as tile
from concourse import mybir
from concourse.bass_utils import run_bass_kernel_spmd
from concourse.masks import make_identity

N_CORES = 8
N, C, CI, H, W = 4, 512, 256, 64, 64
L = H * W          # 4096 tokens
LQ = L // 2        # 2048 queries per core
EPS = 1e-5

F32 = mybir.dt.float32
F32R = mybir.dt.float32r
BF16 = mybir.dt.bfloat16
# matmul operand dtype: F32R (fp32 bits, ~4e-4 rel err) or BF16 (fast
# weight load via FWL, ~8 fewer mantissa bits)
MM_DT = F32R

# module-level flags the test harness may flip
TRACE = False
LAST_RESULTS = None


def r(ap):
    return ap


def build():
    nc = bass.Bass("TRN2", target_bir_lowering=False, debug=False,
                   num_devices=N_CORES)

    x_d = nc.declare_dram_parameter("x", [C, L], MM_DT, isOutput=False)
    tw_d = nc.declare_dram_parameter("tw", [C, CI], MM_DT, isOutput=False)
    pw_d = nc.declare_dram_parameter("pw", [C, CI], MM_DT, isOutput=False)
    gw_d = nc.declare_dram_parameter("gw", [C, CI], MM_DT, isOutput=False)
    zw_d = nc.declare_dram_parameter("zw", [CI, C], MM_DT, isOutput=False)
    tb_d = nc.declare_dram_parameter("tb", [CI, 1], F32, isOutput=False)
    pb_d = nc.declare_dram_parameter("pb", [CI, 1], F32, isOutput=False)
    zb_d = nc.declare_dram_parameter("zb", [C, 1], F32, isOutput=False)
    wy_d = nc.declare_dram_parameter("wy", [C, LQ], F32, isOutput=True)
    st_d = nc.declare_dram_parameter("st", [C, 2], F32, isOutput=True)

    GTW = 264  # padded row width of one gT m-tile (256 ci + ones col + pad)

    with tile.TileContext(nc) as tc, ExitStack() as ctx:
        consts = ctx.enter_context(tc.tile_pool(name="consts", bufs=1))
        projp = ctx.enter_context(tc.tile_pool(name="projout", bufs=1))

        # ---- constants / weights ----
        tw_sb = [consts.tile([128, CI], MM_DT, tag=f"tw{k}", name=f"tw{k}") for k in range(4)]
        pw_sb = [consts.tile([128, CI], MM_DT, tag=f"pw{k}", name=f"pw{k}") for k in range(4)]
        gw_sb = [consts.tile([128, CI], MM_DT, tag=f"gw{k}", name=f"gw{k}") for k in range(4)]
        zw_sb = [consts.tile([128, C], MM_DT, tag=f"zw{k}", name=f"zw{k}") for k in range(2)]
        for k in range(4):
            nc.sync.dma_start(out=tw_sb[k], in_=tw_d[128 * k:128 * k + 128, :])
            nc.sync.dma_start(out=pw_sb[k], in_=pw_d[128 * k:128 * k + 128, :])
            nc.sync.dma_start(out=gw_sb[k], in_=gw_d[128 * k:128 * k + 128, :])
        zw_dma_todo = list(range(2))  # issued after the x chunks (zw is
        # only needed by the first group tail, ~50us in; keeping its 0.5MB
        # out of the chunk-0 window lands chunk 0 ~2us earlier)
        tb_sb = [consts.tile([128, 1], F32, tag=f"tb{i}", name=f"tb{i}") for i in range(2)]
        pb_sb = [consts.tile([128, 1], F32, tag=f"pb{i}", name=f"pb{i}") for i in range(2)]
        zb_sb = [consts.tile([128, 1], F32, tag=f"zb{i}", name=f"zb{i}") for i in range(4)]
        for i in range(2):
            nc.sync.dma_start(out=tb_sb[i], in_=tb_d[128 * i:128 * i + 128, :])
            nc.sync.dma_start(out=pb_sb[i], in_=pb_d[128 * i:128 * i + 128, :])
        for i in range(4):
            nc.sync.dma_start(out=zb_sb[i], in_=zb_d[128 * i:128 * i + 128, :])
        ident = consts.tile([128, 128], F32)
        make_identity(nc, ident)

        # ---- projection outputs (live through phase 2) ----
        th_sb = [projp.tile([128, LQ], MM_DT, tag=f"th{i}", name=f"th{i}") for i in range(2)]
        ph_sb = [projp.tile([128, L], MM_DT, tag=f"ph{i}", name=f"ph{i}") for i in range(2)]
        gt_sb = projp.tile([128, 32, GTW], MM_DT, tag="gt")
        # ones column for the softmax-denominator trick (memset can't write
        # f32r; bounce through an f32 tile and let the DVE copy round)
        ones_c = consts.tile([128, 32, 2], F32, tag="ones", name="ones")
        nc.vector.memset(ones_c, 1.0)
        nc.vector.tensor_copy(out=gt_sb[:, :, 256:258], in_=ones_c)

        # ---- phase 1: projections (x resident only here) ----
        with tc.tile_pool(name="xp", bufs=1) as xpool, \
             tc.tile_pool(name="pproj", bufs=8, space="PSUM") as pproj:
            from concourse.tile import add_dep_helper

            xk = [xpool.tile([128, L], MM_DT, tag=f"x{k}", name=f"x{k}") for k in range(4)]

            # warmup matmuls: run while the PE waits for the first x chunk
            # and keep the HAM activity monitor from clock-throttling the
            # real phase-1 matmuls
            warm_src = xpool.tile([128, 512], MM_DT, tag="warm", name="warm")
            nc.vector.memset(warm_src.bitcast(F32), 0.0)
            wps = pproj.tile([128, 512], F32, tag="proj", name="warmps")
            for _w in range(60):
                nc.tensor.matmul(wps, warm_src[:, 0:128], warm_src,
                                 start=True, stop=True)

            # x streams in COLUMN chunks (all 512 channel rows x 1024 tokens):
            # projections only need the x columns of the tokens they produce,
            # so each chunk's theta/phi/gT matmuls run while the next chunk's
            # DMA is in flight. Chunks are chained so the in-flight chunk gets
            # the full DMA bandwidth (concurrent transfers share it evenly and
            # would all land at ~45us; chained, chunk 0 lands at ~17us).
            prev_dmas = []
            for c in range(4):
                o = 1024 * c
                cur = []
                for k in range(4):
                    for h in range(2):
                        oo = o + 512 * h
                        d = nc.sync.dma_start(out=xk[k][:, oo:oo + 512],
                                              in_=x_d[128 * k:128 * k + 128,
                                                      oo:oo + 512])
                        for pd in prev_dmas:
                            add_dep_helper(d.ins, pd.ins, sync=True,
                                           reason="serialize x col-chunks")
                        cur.append(d)
                prev_dmas = cur
                if c == 3:
                    for k in zw_dma_todo:
                        d = nc.sync.dma_start(
                            out=zw_sb[k], in_=zw_d[128 * k:128 * k + 128, :])

                # phi for this chunk's 1024 tokens: 2 ci-tiles x 2 n-chunks
                ps = [pproj.tile([128, 512], F32, tag="proj", name=f"prph{_i}")
                      for _i in range(4)]
                for k in range(4):
                    for ci_t in range(2):
                        lhs = pw_sb[k][:, 128 * ci_t:128 * ci_t + 128]
                        for nch in range(2):
                            oo = o + 512 * nch
                            nc.tensor.matmul(
                                ps[2 * ci_t + nch], r(lhs),
                                r(xk[k][:, oo:oo + 512]),
                                start=(k == 0), stop=(k == 3))
                for ci_t in range(2):
                    for nch in range(2):
                        oo = o + 512 * nch
                        nc.scalar.activation(
                            out=ph_sb[ci_t][:, oo:oo + 512],
                            in_=ps[2 * ci_t + nch],
                            func=mybir.ActivationFunctionType.Identity,
                            bias=pb_sb[ci_t], scale=1.0)

                # theta for this chunk (queries live in columns 0..2047)
                if c < 2:
                    ps = [pproj.tile([128, 512], F32, tag="proj",
                                     name=f"prth{_i}") for _i in range(4)]
                    for k in range(4):
                        for ci_t in range(2):
                            lhs = tw_sb[k][:, 128 * ci_t:128 * ci_t + 128]
                            for nch in range(2):
                                oo = o + 512 * nch
                                nc.tensor.matmul(
                                    ps[2 * ci_t + nch], r(lhs),
                                    r(xk[k][:, oo:oo + 512]),
                                    start=(k == 0), stop=(k == 3))
                    for ci_t in range(2):
                        for nch in range(2):
                            oo = o + 512 * nch
                            nc.scalar.activation(
                                out=th_sb[ci_t][:, oo:oo + 512],
                                in_=ps[2 * ci_t + nch],
                                func=mybir.ActivationFunctionType.Identity,
                                bias=tb_sb[ci_t], scale=1.0)

                # gT for this chunk's 8 m-tiles (transposed layout, bias
                # folded into zb on the host)
                for mb in range(2):
                    ps = [pproj.tile([128, 512], F32, tag="proj",
                                     name=f"prg{_i}") for _i in range(4)]
                    for k in range(4):
                        for j in range(4):
                            m0 = o + 128 * (4 * mb + j)
                            nc.tensor.matmul(
                                ps[j][:, 0:256], r(xk[k][:, m0:m0 + 128]),
                                r(gw_sb[k]), start=(k == 0), stop=(k == 3))
                    for j in range(4):
                        nc.vector.tensor_copy(
                            out=gt_sb[:, 8 * c + 4 * mb + j, 0:256],
                            in_=ps[j][:, 0:256])

        # ---- phase 2: attention + wz + stats, per 512-query group ----
        p_pt = ctx.enter_context(tc.tile_pool(name="ptbuf", bufs=1))
        p_ps = ctx.enter_context(tc.tile_pool(name="ps", bufs=4, space="PSUM"))
        p_py = ctx.enter_context(tc.tile_pool(name="py", bufs=1, space="PSUM"))
        p_yt = ctx.enter_context(tc.tile_pool(name="yt", bufs=4))
        p_y = ctx.enter_context(tc.tile_pool(name="y", bufs=4))
        p_wy = ctx.enter_context(tc.tile_pool(name="wy", bufs=4))
        p_sm = ctx.enter_context(tc.tile_pool(name="small", bufs=8))
        p_st = ctx.enter_context(tc.tile_pool(name="stats", bufs=1))

        pt_sb = p_pt.tile([128, 32, 512], MM_DT, tag="pt")
        st_sb = [p_st.tile([128, 4, 6], F32, tag=f"bst{c_t}", name=f"bst{c_t}") for c_t in range(4)]

        p_yr = ctx.enter_context(tc.tile_pool(name="yraw", bufs=2))

        def emit_av(py, m_t):
            # y^T[q, ci] accumulation (+ denominator in col 256)
            for qs in range(4):
                nc.tensor.matmul(
                    py[qs],
                    r(pt_sb[:, m_t, 128 * qs:128 * qs + 128]),
                    r(gt_sb[:, m_t, 0:258]),
                    start=(m_t == 0), stop=(m_t == 31))

        def tail_copy(py):
            # drain the finished group's y^T accumulators out of PSUM on the
            # (otherwise idle) DVE so the banks free up for the next group
            yraw = []
            for qs in range(4):
                t = p_yr.tile([128, 258], F32, tag=f"yraw{qs}",
                              name=f"yraw{qs}")
                nc.vector.tensor_copy(out=t, in_=py[qs])
                yraw.append(t)
            return yraw

        def tail_rest(g, q0, yraw):
            # normalize by the col-256 denominator, transpose to [ci, q],
            # wz-project, bias, batch-norm stats, store
            yt = []
            for qs in range(4):
                rs = p_sm.tile([128, 1], F32, tag="rs")
                nc.vector.reciprocal(out=rs, in_=yraw[qs][:, 256:257])
                t = p_yt.tile([128, 256], F32, tag="yt")
                nc.scalar.mul(t, yraw[qs][:, 0:256], rs)
                yt.append(t)
            y_sb = []
            for ci_t in range(2):
                pt2 = p_ps.tile([128, 512], F32, tag="s")
                for qs in range(4):
                    nc.tensor.transpose(
                        pt2[:, 128 * qs:128 * qs + 128],
                        yt[qs][:, 128 * ci_t:128 * ci_t + 128],
                        ident)
                t = p_y.tile([128, 512], MM_DT, tag="y")
                nc.scalar.copy(t, pt2)
                y_sb.append(t)
            for c_t in range(4):
                pw_ = p_ps.tile([128, 512], F32, tag="s")
                for k in range(2):
                    nc.tensor.matmul(
                        pw_, r(zw_sb[k][:, 128 * c_t:128 * c_t + 128]),
                        r(y_sb[k]), start=(k == 0), stop=(k == 1))
                wt = p_wy.tile([128, 512], F32, tag="wy")
                nc.scalar.activation(
                    out=wt, in_=pw_,
                    func=mybir.ActivationFunctionType.Identity,
                    bias=zb_sb[c_t], scale=1.0)
                nc.vector.bn_stats(out=st_sb[c_t][:, g, :], in_=wt)
                nc.sync.dma_start(
                    out=wy_d[128 * c_t:128 * c_t + 128, q0:q0 + 512], in_=wt)

        # S^T tile -> exp -> P^T, with the previous m-tile's AV matmuls
        # interleaved so the PE never waits on the ACT exp, and the previous
        # GROUP's normalize/transpose/wz tail skewed into the first few
        # m-tiles of the current group so the PE never waits on it either
        prev = None
        for g in range(4):
            q0 = 512 * g
            py = [p_py.tile([128, 258], F32, tag=f"py{qs}", name=f"pyt{qs}")
                  for qs in range(4)]
            for m_t in range(32):
                ps = p_ps.tile([128, 512], F32, tag="s")
                for ci_t in range(2):
                    nc.tensor.matmul(
                        ps,
                        r(ph_sb[ci_t][:, 128 * m_t:128 * m_t + 128]),
                        r(th_sb[ci_t][:, q0:q0 + 512]),
                        start=(ci_t == 0), stop=(ci_t == 1))
                nc.scalar.activation(
                    out=pt_sb[:, m_t, :], in_=ps,
                    func=mybir.ActivationFunctionType.Exp)
                if m_t == 0 and prev is not None:
                    prev_yraw = tail_copy(prev[2])
                if m_t >= 1:
                    emit_av(py, m_t - 1)
                if m_t == 2 and prev is not None:
                    tail_rest(prev[0], prev[1], prev_yraw)
            emit_av(py, 31)
            prev = (g, q0, py)
        tail_rest(prev[0], prev[1], tail_copy(prev[2]))

        # aggregate stats -> [mean, var] per channel
        for c_t in range(4):
            mv = p_sm.tile([128, 2], F32, tag="mv")
            nc.vector.bn_aggr(out=mv, in_=st_sb[c_t])
            nc.sync.dma_start(out=st_d[128 * c_t:128 * c_t + 128, :], in_=mv)

    _split_waits(nc)
    return nc


def _split_waits(nc, max_waits=1):
    """walrus in this container only encodes 1 sem wait per instruction;
    hoist overflow waits onto preceding same-engine NOPs."""
    n = 0
    for f in nc.m.functions:
        for bb in f.blocks:
            changed = False
            out = []
            for ins in bb.instructions:
                si = ins.sync_info
                if si is not None and si.on_wait and len(si.on_wait) > max_waits:
                    waits = list(si.on_wait)
                    while len(waits) > max_waits:
                        chunk, waits = waits[:max_waits], waits[max_waits:]
                        n += 1
                        out.append(mybir.InstNoOp(
                            name=f"I-waitsplit-{n}", engine=ins.engine,
                            sync_info=mybir.SyncInfo(on_wait=chunk, on_update=[])))
                    ins.sync_info = mybir.SyncInfo(
                        on_wait=waits, on_update=list(si.on_update))
                    changed = True
                out.append(ins)
            if changed:
                bb.instructions = out
    return n


_NC = None


def _get_nc():
    global _NC
    if _NC is None:
        _NC = build()
    return _NC


def kernel(x, g_w, g_b, theta_w, theta_b, phi_w, phi_b, wz_w, wz_b,
           bn_gamma, bn_beta):
    global LAST_RESULTS
    from concourse.dt import dt as _dt

    np_mm = _dt.np(MM_DT)
    x = np.asarray(x, dtype=np.float32)
    tw = np.ascontiguousarray(np.asarray(theta_w, np.float32).T).astype(np_mm)
    pw = np.ascontiguousarray(np.asarray(phi_w, np.float32).T).astype(np_mm)
    gw = np.ascontiguousarray(np.asarray(g_w, np.float32).T).astype(np_mm)
    zw = np.ascontiguousarray(np.asarray(wz_w, np.float32).T).astype(np_mm)
    tb = np.asarray(theta_b, np.float32).reshape(CI, 1)
    pb = np.asarray(phi_b, np.float32).reshape(CI, 1)
    zb = (np.asarray(wz_b, np.float32)
          + np.asarray(wz_w, np.float32) @ np.asarray(g_b, np.float32)
          ).reshape(C, 1)

    xf = x.reshape(N, C, L)
    in_maps = []
    for core in range(N_CORES):
        n, half = divmod(core, 2)
        xn = xf[n]
        if half:
            xn = np.concatenate([xn[:, LQ:], xn[:, :LQ]], axis=1)
        xn = np.ascontiguousarray(xn).astype(np_mm)
        in_maps.append({
            "x": xn, "tw": tw, "pw": pw, "gw": gw, "zw": zw,
            "tb": tb, "pb": pb, "zb": zb,
        })

    nc = _get_nc()
    res = run_bass_kernel_spmd(nc, in_maps, list(range(N_CORES)), trace=TRACE)
    LAST_RESULTS = res

    wy = np.empty((N, C, L), dtype=np.float32)
    means = np.empty((N_CORES, C), dtype=np.float32)
    varis = np.empty((N_CORES, C), dtype=np.float32)
    for core in range(N_CORES):
        n, half = divmod(core, 2)
        wy[n, :, half * LQ:(half + 1) * LQ] = res.results[core]["wy"]
        means[core] = res.results[core]["st"][:, 0]
        varis[core] = res.results[core]["st"][:, 1]

    m = means.mean(axis=0)
    v = (varis + means ** 2).mean(axis=0) - m ** 2
    scale = (np.asarray(bn_gamma, np.float32)
             / np.sqrt(v + EPS)).astype(np.float32)
    shift = (np.asarray(bn_beta, np.float32) - m * scale).astype(np.float32)
    out = wy * scale[None, :, None] + shift[None, :, None] + xf
    return out.reshape(N, C, H, W)


# revision 28
# speedup vs baseline: 1.0359x; 1.0359x over previous
"""NLBlockND (embedded-gaussian non-local block, 2D, bn_layer=True) on 8 TRN2 cores.

Strategy (see spec sharding hint): data-parallel over batch N=4, x2
sequence-parallel over the 4096 query tokens -> 8 shards of (batch n,
query half h). Each core gets the full 512x4096 x for its batch with the
key/query axis ROTATED so its 2048 queries are always columns 0..2047
(attention is permutation-invariant over keys, so rotating the key axis
changes nothing). Per core:

  theta = theta_w^T-proj of x[:, :2048] + tb          [256, 2048]  (f32r matmul)
  phi   = phi_w^T-proj of x + pb                      [256, 4096]
  gT    = x^T-proj against g_w^T (transposed layout)  [4096, 256] (+ ones col)
  S^T   = phi_tile^T @ theta  (keys on partitions)    [4096, 512] per q-group
  P^T   = exp(S^T)            (no max-sub needed: |logit| <= ~20)
  y^T   = P^T-tiles^T @ [gT | 1]  -> cols 0..255 = unnormalized y^T,
          col 256 = softmax denominator s[q]  (free row-sum trick)
  y^T  *= 1/s[q]  (per-partition scale fused into PSUM->SBUF copy)
  y     = PE-transpose(y^T)
  w_y   = wz_w^T-proj of y + (wz_b + wz_w @ g_b)      [512, 2048]
          (g_b is folded into the wz bias on the host: attn rows sum to 1)
  per-channel mean/var of w_y slice via bn_stats/bn_aggr

Host: gather w_y slices, combine slice stats into global batch-norm
stats, normalize, affine, residual-add x.

All big matmuls run as float32r (fp32 bits, full PE rate at N>=256).
"""

import sys

if "/opt/trn_rl_repo" not in sys.path:
    sys.path.insert(0, "/opt/trn_rl_repo")

from contextlib import ExitStack

import numpy as np

import concourse.bass as bass
import concourse.tile as tile
from concourse import mybir
from concourse.bass_utils import run_bass_kernel_spmd
from concourse.masks import make_identity

N_CORES = 8
N, C, CI, H, W = 4, 512, 256, 64, 64
L = H * W          # 4096 tokens
LQ = L // 2        # 2048 queries per core
EPS = 1e-5

F32 = mybir.dt.float32
F32R = mybir.dt.float32r
BF16 = mybir.dt.bfloat16
# matmul operand dtype: F32R (fp32 bits, ~4e-4 rel err) or BF16 (fast
# weight load via FWL, ~8 fewer mantissa bits)
MM_DT = F32R

# module-level flags the test harness may flip
TRACE = False
LAST_RESULTS = None


def r(ap):
    return ap


def build():
    nc = bass.Bass("TRN2", target_bir_lowering=False, debug=False,
                   num_devices=N_CORES)

    x_d = nc.declare_dram_parameter("x", [C, L], MM_DT, isOutput=False)
    tw_d = nc.declare_dram_parameter("tw", [C, CI], MM_DT, isOutput=False)
    pw_d = nc.declare_dram_parameter("pw", [C, CI], MM_DT, isOutput=False)
    gw_d = nc.declare_dram_parameter("gw", [C, CI], MM_DT, isOutput=False)
    zw_d = nc.declare_dram_parameter("zw", [CI, C], MM_DT, isOutput=False)
    tb_d = nc.declare_dram_parameter("tb", [CI, 1], F32, isOutput=False)
    pb_d = nc.declare_dram_parameter("pb", [CI, 1], F32, isOutput=False)
    zb_d = nc.declare_dram_parameter("zb", [C, 1], F32, isOutput=False)
    wy_d = nc.declare_dram_parameter("wy", [C, LQ], F32, isOutput=True)
    st_d = nc.declare_dram_parameter("st", [C, 2], F32, isOutput=True)

    GTW = 264  # padded row width of one gT m-tile (256 ci + ones col + pad)

    with tile.TileContext(nc) as tc, ExitStack() as ctx:
        consts = ctx.enter_context(tc.tile_pool(name="consts", bufs=1))
        projp = ctx.enter_context(tc.tile_pool(name="projout", bufs=1))

        # ---- constants / weights ----
        tw_sb = [consts.tile([128, CI], MM_DT, tag=f"tw{k}", name=f"tw{k}") for k in range(4)]
        pw_sb = [consts.tile([128, CI], MM_DT, tag=f"pw{k}", name=f"pw{k}") for k in range(4)]
        gw_sb = [consts.tile([128, CI], MM_DT, tag=f"gw{k}", name=f"gw{k}") for k in range(4)]
        zw_sb = [consts.tile([128, C], MM_DT, tag=f"zw{k}", name=f"zw{k}") for k in range(2)]
        for k in range(4):
            nc.sync.dma_start(out=tw_sb[k], in_=tw_d[128 * k:128 * k + 128, :])
            nc.sync.dma_start(out=pw_sb[k], in_=pw_d[128 * k:128 * k + 128, :])
            nc.sync.dma_start(out=gw_sb[k], in_=gw_d[128 * k:128 * k + 128, :])
        zw_dma_todo = list(range(2))  # issued after the x chunks (zw is
        # only needed by the first group tail, ~50us in; keeping its 0.5MB
        # out of the chunk-0 window lands chunk 0 ~2us earlier)
        tb_sb = [consts.tile([128, 1], F32, tag=f"tb{i}", name=f"tb{i}") for i in range(2)]
        pb_sb = [consts.tile([128, 1], F32, tag=f"pb{i}", name=f"pb{i}") for i in range(2)]
        zb_sb = [consts.tile([128, 1], F32, tag=f"zb{i}", name=f"zb{i}") for i in range(4)]
        for i in range(2):
            nc.sync.dma_start(out=tb_sb[i], in_=tb_d[128 * i:128 * i + 128, :])
            nc.sync.dma_start(out=pb_sb[i], in_=pb_d[128 * i:128 * i + 128, :])
        for i in range(4):
            nc.sync.dma_start(out=zb_sb[i], in_=zb_d[128 * i:128 * i + 128, :])
        ident = consts.tile([128, 128], F32)
        make_identity(nc, ident)

        # ---- projection outputs (live through phase 2) ----
        th_sb = [projp.tile([128, LQ], MM_DT, tag=f"th{i}", name=f"th{i}") for i in range(2)]
        ph_sb = [projp.tile([128, L], MM_DT, tag=f"ph{i}", name=f"ph{i}") for i in range(2)]
        gt_sb = projp.tile([128, 32, GTW], MM_DT, tag="gt")
        # ones column for the softmax-denominator trick (memset can't write
        # f32r; bounce through an f32 tile and let the DVE copy round)
        ones_c = consts.tile([128, 32, 2], F32, tag="ones", name="ones")
        nc.vector.memset(ones_c, 1.0)
        nc.vector.tensor_copy(out=gt_sb[:, :, 256:258], in_=ones_c)

        # ---- phase 1: projections (x resident only here) ----
        with tc.tile_pool(name="xp", bufs=1) as xpool, \
             tc.tile_pool(name="pproj", bufs=8, space="PSUM") as pproj:
            from concourse.tile import add_dep_helper

            xk = [xpool.tile([128, L], MM_DT, tag=f"x{k}", name=f"x{k}") for k in range(4)]

            # warmup matmuls: run while the PE waits for the first x chunk
            # and keep the HAM activity monitor from clock-throttling the
            # real phase-1 matmuls
            warm_src = xpool.tile([128, 512], MM_DT, tag="warm", name="warm")
            nc.vector.memset(warm_src.bitcast(F32), 0.0)
            wps = pproj.tile([128, 512], F32, tag="proj", name="warmps")
            for _w in range(60):
                nc.tensor.matmul(wps, warm_src[:, 0:128], warm_src,
                                 start=True, stop=True)

            # x streams in COLUMN chunks (all 512 channel rows x 1024 tokens):
            # projections only need the x columns of the tokens they produce,
            # so each chunk's theta/phi/gT matmuls run while the next chunk's
            # DMA is in flight. Chunks are chained so the in-flight chunk gets
            # the full DMA bandwidth (concurrent transfers share it evenly and
            # would all land at ~45us; chained, chunk 0 lands at ~17us).
            prev_dmas = []
            for c in range(4):
                o = 1024 * c
                cur = []
                for k in range(4):
                    for h in range(2):
                        oo = o + 512 * h
                        d = nc.sync.dma_start(out=xk[k][:, oo:oo + 512],
                                              in_=x_d[128 * k:128 * k + 128,
                                                      oo:oo + 512])
                        for pd in prev_dmas:
                            add_dep_helper(d.ins, pd.ins, sync=True,
                                           reason="serialize x col-chunks")
                        cur.append(d)
                prev_dmas = cur
                if c == 3:
                    for k in zw_dma_todo:
                        d = nc.sync.dma_start(
                            out=zw_sb[k], in_=zw_d[128 * k:128 * k + 128, :])

                # phi for this chunk's 1024 tokens: 2 ci-tiles x 2 n-chunks
                ps = [pproj.tile([128, 512], F32, tag="proj", name=f"prph{_i}")
                      for _i in range(4)]
                for k in range(4):
                    for ci_t in range(2):
                        lhs = pw_sb[k][:, 128 * ci_t:128 * ci_t + 128]
                        for nch in range(2):
                            oo = o + 512 * nch
                            nc.tensor.matmul(
                                ps[2 * ci_t + nch], r(lhs),
                                r(xk[k][:, oo:oo + 512]),
                                start=(k == 0), stop=(k == 3))
                for ci_t in range(2):
                    for nch in range(2):
                        oo = o + 512 * nch
                        nc.scalar.activation(
                            out=ph_sb[ci_t][:, oo:oo + 512],
                            in_=ps[2 * ci_t + nch],
                            func=mybir.ActivationFunctionType.Identity,
                            bias=pb_sb[ci_t], scale=1.0)

                # theta for this chunk (queries live in columns 0..2047)
                if c < 2:
                    ps = [pproj.tile([128, 512], F32, tag="proj",
                                     name=f"prth{_i}") for _i in range(4)]
                    for k in range(4):
                        for ci_t in range(2):
                            lhs = tw_sb[k][:, 128 * ci_t:128 * ci_t + 128]
                            for nch in range(2):
                                oo = o + 512 * nch
                                nc.tensor.matmul(
                                    ps[2 * ci_t + nch], r(lhs),
                                    r(xk[k][:, oo:oo + 512]),
                                    start=(k == 0), stop=(k == 3))
                    for ci_t in range(2):
                        for nch in range(2):
                            oo = o + 512 * nch
                            nc.scalar.activation(
                                out=th_sb[ci_t][:, oo:oo + 512],
                                in_=ps[2 * ci_t + nch],
                                func=mybir.ActivationFunctionType.Identity,
                                bias=tb_sb[ci_t], scale=1.0)

                # gT for this chunk's 8 m-tiles (transposed layout, bias
                # folded into zb on the host)
                for mb in range(2):
                    ps = [pproj.tile([128, 512], F32, tag="proj",
                                     name=f"prg{_i}") for _i in range(4)]
                    for k in range(4):
                        for j in range(4):
                            m0 = o + 128 * (4 * mb + j)
                            nc.tensor.matmul(
                                ps[j][:, 0:256], r(xk[k][:, m0:m0 + 128]),
                                r(gw_sb[k]), start=(k == 0), stop=(k == 3))
                    for j in range(4):
                        nc.vector.tensor_copy(
                            out=gt_sb[:, 8 * c + 4 * mb + j, 0:256],
                            in_=ps[j][:, 0:256])

        # ---- phase 2: attention + wz + stats, per 512-query group ----
        p_pt = ctx.enter_context(tc.tile_pool(name="ptbuf", bufs=1))
        p_ps = ctx.enter_context(tc.tile_pool(name="ps", bufs=3, space="PSUM"))
        p_pm = ctx.enter_context(tc.tile_pool(name="pmisc", bufs=1, space="PSUM"))
        p_py = ctx.enter_context(tc.tile_pool(name="py", bufs=1, space="PSUM"))
        p_yt = ctx.enter_context(tc.tile_pool(name="yt", bufs=4))
        p_y = ctx.enter_context(tc.tile_pool(name="y", bufs=4))
        p_wy = ctx.enter_context(tc.tile_pool(name="wy", bufs=4))
        p_sm = ctx.enter_context(tc.tile_pool(name="small", bufs=8))
        p_st = ctx.enter_context(tc.tile_pool(name="stats", bufs=1))

        pt_sb = p_pt.tile([128, 32, 512], MM_DT, tag="pt")
        st_sb = [p_st.tile([128, 4, 6], F32, tag=f"bst{c_t}", name=f"bst{c_t}") for c_t in range(4)]

        p_yr = ctx.enter_context(tc.tile_pool(name="yraw", bufs=2))

        def emit_av(py, m_t):
            # y^T[q, ci] accumulation (+ denominator in col 256)
            for qs in range(4):
                nc.tensor.matmul(
                    py[qs],
                    r(pt_sb[:, m_t, 128 * qs:128 * qs + 128]),
                    r(gt_sb[:, m_t, 0:258]),
                    start=(m_t == 0), stop=(m_t == 31))

        def tail_copy(py):
            # drain the finished group's y^T accumulators out of PSUM on the
            # (otherwise idle) DVE so the banks free up for the next group
            yraw = []
            for qs in range(4):
                t = p_yr.tile([128, 258], F32, tag=f"yraw{qs}",
                              name=f"yraw{qs}")
                nc.vector.tensor_copy(out=t, in_=py[qs])
                yraw.append(t)
            return yraw

        def tail_rest(g, q0, yraw):
            # normalize by the col-256 denominator, transpose to [ci, q],
            # wz-project, bias, batch-norm stats, store
            yt = []
            for qs in range(4):
                rs = p_sm.tile([128, 1], F32, tag="rs")
                nc.vector.reciprocal(out=rs, in_=yraw[qs][:, 256:257])
                t = p_yt.tile([128, 256], F32, tag="yt")
                nc.scalar.mul(t, yraw[qs][:, 0:256], rs)
                yt.append(t)
            y_sb = []
            for ci_t in range(2):
                pt2 = p_ps.tile([128, 512], F32, tag="s")
                for qs in range(4):
                    nc.tensor.transpose(
                        pt2[:, 128 * qs:128 * qs + 128],
                        yt[qs][:, 128 * ci_t:128 * ci_t + 128],
                        ident)
                t = p_y.tile([128, 512], MM_DT, tag="y")
                nc.scalar.copy(t, pt2)
                y_sb.append(t)
            for c_t in range(4):
                pw_ = p_pm.tile([128, 512], F32, tag="pm")
                for k in range(2):
                    nc.tensor.matmul(
                        pw_, r(zw_sb[k][:, 128 * c_t:128 * c_t + 128]),
                        r(y_sb[k]), start=(k == 0), stop=(k == 1))
                wt = p_wy.tile([128, 512], F32, tag="wy")
                nc.scalar.activation(
                    out=wt, in_=pw_,
                    func=mybir.ActivationFunctionType.Identity,
                    bias=zb_sb[c_t], scale=1.0)
                nc.vector.bn_stats(out=st_sb[c_t][:, g, :], in_=wt)
                nc.sync.dma_start(
                    out=wy_d[128 * c_t:128 * c_t + 128, q0:q0 + 512], in_=wt)

        # S^T tile -> exp -> P^T, with the previous m-tile's AV matmuls
        # interleaved so the PE never waits on the ACT exp, and the previous
        # GROUP's normalize/transpose/wz tail skewed into the first few
        # m-tiles of the current group so the PE never waits on it either
        prev = None
        for g in range(4):
            q0 = 512 * g
            py = [p_py.tile([128, 258], F32, tag=f"py{qs}", name=f"pyt{qs}")
                  for qs in range(4)]
            for m_t in range(32):
                ps = p_ps.tile([128, 512], F32, tag="s")
                for ci_t in range(2):
                    nc.tensor.matmul(
                        ps,
                        r(ph_sb[ci_t][:, 128 * m_t:128 * m_t + 128]),
                        r(th_sb[ci_t][:, q0:q0 + 512]),
                        start=(ci_t == 0), stop=(ci_t == 1))
                nc.scalar.activation(
                    out=pt_sb[:, m_t, :], in_=ps,
                    func=mybir.ActivationFunctionType.Exp)
                if m_t == 0 and prev is not None:
                    prev_yraw = tail_copy(prev[2])
                if m_t >= 1:
                    emit_av(py, m_t - 1)
                if m_t == 2 and prev is not None:
                    tail_rest(prev[0], prev[1], prev_yraw)
            emit_av(py, 31)
            prev = (g, q0, py)
        tail_rest(prev[0], prev[1], tail_copy(prev[2]))

        # aggregate stats -> [mean, var] per channel
        for c_t in range(4):
            mv = p_sm.tile([128, 2], F32, tag="mv")
            nc.vector.bn_aggr(out=mv, in_=st_sb[c_t])
            nc.sync.dma_start(out=st_d[128 * c_t:128 * c_t + 128, :], in_=mv)

    _split_waits(nc)
    return nc


def _split_waits(nc, max_waits=1):
    """walrus in this container only encodes 1 sem wait per instruction;
    hoist overflow waits onto preceding same-engine NOPs."""
    n = 0
    for f in nc.m.functions:
        for bb in f.blocks:
            changed = False
            out = []
            for ins in bb.instructions:
                si = ins.sync_info
                if si is not None and si.on_wait and len(si.on_wait) > max_waits:
                    waits = list(si.on_wait)
                    while len(waits) > max_waits:
                        chunk, waits = waits[:max_waits], waits[max_waits:]
                        n += 1
                        out.append(mybir.InstNoOp(
                            name=f"I-waitsplit-{n}", engine=ins.engine,
                            sync_info=mybir.SyncInfo(on_wait=chunk, on_update=[])))
                    ins.sync_info = mybir.SyncInfo(
                        on_wait=waits, on_update=list(si.on_update))
                    changed = True
                out.append(ins)
            if changed:
                bb.instructions = out
    return n


_NC = None


def _get_nc():
    global _NC
    if _NC is None:
        _NC = build()
    return _NC


def kernel(x, g_w, g_b, theta_w, theta_b, phi_w, phi_b, wz_w, wz_b,
           bn_gamma, bn_beta):
    global LAST_RESULTS
    from concourse.dt import dt as _dt

    np_mm = _dt.np(MM_DT)
    x = np.asarray(x, dtype=np.float32)
    tw = np.ascontiguousarray(np.asarray(theta_w, np.float32).T).astype(np_mm)
    pw = np.ascontiguousarray(np.asarray(phi_w, np.float32).T).astype(np_mm)
    gw = np.ascontiguousarray(np.asarray(g_w, np.float32).T).astype(np_mm)
    zw = np.ascontiguousarray(np.asarray(wz_w, np.float32).T).astype(np_mm)
    tb = np.asarray(theta_b, np.float32).reshape(CI, 1)
    pb = np.asarray(phi_b, np.float32).reshape(CI, 1)
    zb = (np.asarray(wz_b, np.float32)
          + np.asarray(wz_w, np.float32) @ np.asarray(g_b, np.float32)
          ).reshape(C, 1)

    xf = x.reshape(N, C, L)
    in_maps = []
    for core in range(N_CORES):
        n, half = divmod(core, 2)
        xn = xf[n]
        if half:
            xn = np.concatenate([xn[:, LQ:], xn[:, :LQ]], axis=1)
        xn = np.ascontiguousarray(xn).astype(np_mm)
        in_maps.append({
            "x": xn, "tw": tw, "pw": pw, "gw": gw, "zw": zw,
            "tb": tb, "pb": pb, "zb": zb,
        })

    nc = _get_nc()
    res = run_bass_kernel_spmd(nc, in_maps, list(range(N_CORES)), trace=TRACE)
    LAST_RESULTS = res

    wy = np.empty((N, C, L), dtype=np.float32)
    means = np.empty((N_CORES, C), dtype=np.float32)
    varis = np.empty((N_CORES, C), dtype=np.float32)
    for core in range(N_CORES):
        n, half = divmod(core, 2)
        wy[n, :, half * LQ:(half + 1) * LQ] = res.results[core]["wy"]
        means[core] = res.results[core]["st"][:, 0]
        varis[core] = res.results[core]["st"][:, 1]

    m = means.mean(axis=0)
    v = (varis + means ** 2).mean(axis=0) - m ** 2
    scale = (np.asarray(bn_gamma, np.float32)
             / np.sqrt(v + EPS)).astype(np.float32)
    shift = (np.asarray(bn_beta, np.float32) - m * scale).astype(np.float32)
    out = wy * scale[None, :, None] + shift[None, :, None] + xf
    return out.reshape(N, C, H, W)
